# revision 1
# baseline (speedup 1.0000x reference)
"""Complex multi-head attention (RoPE, causal, Hermitian scores) on 8 trn2 cores.

Sharding: core = b*4 + g  (b in {0,1} batches, g in {0..3} head-groups of 4 heads).
Dataflow is fully "transposed" (feature-major) so no on-device transposes are
needed: x is host-transposed to [D, L]; all complex-linear algebra is folded
into host-prepared stacked weight matrices.

Per-core pipeline:
  A1: qTs/kTs projections ([qr;qi] stacked on 128 partitions per head) + RoPE
  A2: v natural [lk, (vr|1|vi|1)] per head (ones column -> softmax rowsum)
  B:  S_T = kTs.T @ qTs per (head, lq-chunk), exp (no max-sub; scores are O(5)),
      O_aug.T = v_aug.T @ P_T accumulated over lk tiles, normalize by rowsum
  C:  out = oT.T @ woT (complex), DMA to DRAM; host sums 4 group partials.
All matmuls run as float32r (full-rate fp32 on PE for moving dim >= 256).
"""

import numpy as np

B, L, D, H = 2, 2048, 1024, 16
HD = 64
G = 4           # head groups (cores per batch)
HPG = 4         # heads per group
GD = HPG * HD   # 256 dims per group
THETA = 10000.0

_compiled = {}


def _rope_tables():
    # cosT2/sinT2: [128, L] tables for the stacked [qr(64); qi(64)] layout.
    j = np.arange(32, dtype=np.float64)
    inv_freq = 1.0 / (THETA ** (2.0 * j / HD))
    freqs = np.arange(L, dtype=np.float64)[:, None] * inv_freq[None, :]  # [L, 32]
    cos = np.cos(freqs).T.astype(np.float32)  # [32, L]
    sin = np.sin(freqs).T.astype(np.float32)
    cos64 = np.concatenate([cos, cos], 0)            # row d: cos[d%32]
    sin64 = np.concatenate([-sin, sin], 0)           # row d<32: -sin, else +sin
    cosT2 = np.concatenate([cos64, cos64], 0)        # stack for qr and qi halves
    sinT2 = np.concatenate([sin64, sin64], 0)
    return np.ascontiguousarray(cosT2), np.ascontiguousarray(sinT2)


def _perm_matrix():
    # PERMT[c, d] = 1 iff c = 64*(d//64) + ((d%64)+32)%64  (lhsT for qsh = P@q)
    p = np.zeros((128, 128), np.float32)
    for d in range(128):
        c = 64 * (d // 64) + ((d % 64) + 32) % 64
        p[c, d] = 1.0
    return p


def _mask_tile():
    # mask2[s]: multiplicative mask for a [128,512] P_T tile whose diagonal
    # 128-block sits at column 128*s: cols<128s are fully-masked (0), the
    # diagonal block is upper-triangular-inclusive, cols beyond are 1.
    out = np.empty((4, 128, 512), np.float32)
    lk = np.arange(128)[:, None]
    f = np.arange(512)[None, :]
    for s in range(4):
        lq = f - 128 * s
        out[s] = np.where(lq < 0, 0.0, (lk <= lq).astype(np.float32))
    return np.ascontiguousarray(out.reshape(512, 512))


def _stack_qk(wr, wi, g):
    # lhsT [1024, 512]: per head block [w_r[rows]; w_i[rows]].T
    blocks = []
    for h in range(HPG):
        r = slice((g * HPG + h) * HD, (g * HPG + h + 1) * HD)
        blocks.append(np.concatenate([wr[r], wi[r]], 0).T)        # [1024, 128]
    return np.ascontiguousarray(np.concatenate(blocks, 1))        # [1024, 512]


def _build_kernel():
    import concourse.bass as bass
    import concourse.mybir as mybir
    import concourse.tile as tile
    from concourse import bacc
    from concourse.bass import ts

    f32 = mybir.dt.float32
    f32r = mybir.dt.float32r
    EXPF = mybir.ActivationFunctionType.Exp

    nc = bacc.Bacc()
    XTR = nc.dram_tensor("xtr", [D, L], f32r, kind="ExternalInput")
    XTI = nc.dram_tensor("xti", [D, L], f32r, kind="ExternalInput")
    WQ1 = nc.dram_tensor("wq1", [D, 512], f32r, kind="ExternalInput")
    WQ2 = nc.dram_tensor("wq2", [D, 512], f32r, kind="ExternalInput")
    WK1 = nc.dram_tensor("wk1", [D, 512], f32r, kind="ExternalInput")
    WK2 = nc.dram_tensor("wk2", [D, 512], f32r, kind="ExternalInput")
    WVA = nc.dram_tensor("wva", [D, 512], f32r, kind="ExternalInput")
    WVB = nc.dram_tensor("wvb", [D, 512], f32r, kind="ExternalInput")
    WOR = nc.dram_tensor("wor", [GD, D], f32r, kind="ExternalInput")
    WOI = nc.dram_tensor("woi", [GD, D], f32r, kind="ExternalInput")
    NWOI = nc.dram_tensor("nwoi", [GD, D], f32r, kind="ExternalInput")
    COS2 = nc.dram_tensor("cos2", [128, L], f32r, kind="ExternalInput")
    SIN2 = nc.dram_tensor("sin2", [128, L], f32r, kind="ExternalInput")
    MASK2 = nc.dram_tensor("mask2", [512, 512], f32, kind="ExternalInput")
    PERM = nc.dram_tensor("perm", [128, 128], f32r, kind="ExternalInput")
    VONES = nc.dram_tensor("vones", [128, 128], f32r, kind="ExternalInput")
    OUTR = nc.dram_tensor("outr", [L, D], f32, kind="ExternalOutput")
    OUTI = nc.dram_tensor("outi", [L, D], f32, kind="ExternalOutput")

    xtr_t = XTR[:, :].rearrange("(ko p) l -> p ko l", p=128)
    xti_t = XTI[:, :].rearrange("(ko p) l -> p ko l", p=128)
    w_t = lambda W: W[:, :].rearrange("(ko p) m -> p ko m", p=128)
    wo_t = lambda W: W[:, :].rearrange("(ko p) d -> p ko d", p=128)

    def mm(out, lhsT, rhs, start, stop):
        nc.tensor.matmul(out, lhsT, rhs, start=start, stop=stop)

    with tile.TileContext(nc) as tc:
        with tc.tile_pool(name="qk", bufs=1) as pQK:
            QTS = pQK.tile([128, HPG, L], f32r, tag="qts")
            KTS = pQK.tile([128, HPG, L], f32r, tag="kts")

            # ---------------- Phase A1: q/k projections + RoPE ----------------
            with (
                tc.tile_pool(name="wqk", bufs=1) as pW,
                tc.tile_pool(name="xin", bufs=2) as pX,
                tc.tile_pool(name="rope", bufs=2) as pR,
                tc.tile_pool(name="psA", bufs=4, space="PSUM") as psA,
                tc.tile_pool(name="psh", bufs=2, space="PSUM") as psH,
            ):
                permt = pW.tile([128, 128], f32r, tag="perm")
                nc.sync.dma_start(permt[:], PERM[:, :])
                wq1 = pW.tile([128, 8, 512], f32r, tag="wq1")
                wq2 = pW.tile([128, 8, 512], f32r, tag="wq2")
                wk1 = pW.tile([128, 8, 512], f32r, tag="wk1")
                wk2 = pW.tile([128, 8, 512], f32r, tag="wk2")
                nc.sync.dma_start(wq1[:], w_t(WQ1))
                nc.sync.dma_start(wq2[:], w_t(WQ2))
                nc.sync.dma_start(wk1[:], w_t(WK1))
                nc.sync.dma_start(wk2[:], w_t(WK2))

                CW = 256  # l-chunk width
                for c in range(L // CW):
                    lsl = ts(c, CW)
                    xr = pX.tile([128, 8, CW], f32r, tag="xr")
                    xi = pX.tile([128, 8, CW], f32r, tag="xi")
                    nc.sync.dma_start(xr[:], xtr_t[:, :, lsl])
                    nc.sync.dma_start(xi[:], xti_t[:, :, lsl])
                    for w1, w2, dest in ((wq1, wq2, QTS), (wk1, wk2, KTS)):
                        for h in range(HPG):
                            ps = psA.tile([128, CW], f32, tag="psA")
                            for kk in range(8):
                                mm(ps[:], w1[:, kk, ts(h, 128)], xr[:, kk, :],
                                   start=(kk == 0), stop=False)
                            for kk in range(8):
                                mm(ps[:], w2[:, kk, ts(h, 128)], xi[:, kk, :],
                                   start=False, stop=(kk == 7))
                            nc.vector.tensor_copy(dest[:, h, lsl], ps[:])
                    # RoPE on this chunk for q and k
                    cosc = pR.tile([128, CW], f32r, tag="cos")
                    sinc = pR.tile([128, CW], f32r, tag="sin")
                    nc.sync.dma_start(cosc[:], COS2[:, lsl])
                    nc.sync.dma_start(sinc[:], SIN2[:, lsl])
                    for dest in (QTS, KTS):
                        # partition rotate by 32 within each 64-block, on PE
                        psh = psH.tile([128, HPG, CW], f32, tag="psh")
                        for j in range(2):
                            mm(psh[:, 2 * j:2 * j + 2, :], permt[:],
                               dest[:, 2 * j:2 * j + 2, lsl],
                               start=True, stop=True)
                        tmp = pR.tile([128, HPG, CW], f32r, tag="tmp")
                        nc.vector.tensor_mul(
                            tmp[:], dest[:, :, lsl],
                            cosc[:, None, :].to_broadcast((128, HPG, CW)))
                        shm = pR.tile([128, HPG, CW], f32r, tag="shm")
                        nc.vector.tensor_mul(
                            shm[:], psh[:],
                            sinc[:, None, :].to_broadcast((128, HPG, CW)))
                        nc.vector.tensor_add(dest[:, :, lsl], tmp[:], shm[:])

            # ---------------- Phase A2: v natural ----------------
            pV = tc.alloc_tile_pool(name="vres", bufs=1)
            VA = pV.tile([128, 16, HPG * 130], f32r, tag="va")
            with (
                tc.tile_pool(name="wv", bufs=1) as pWv,
                tc.tile_pool(name="xin2", bufs=2) as pX2,
                tc.tile_pool(name="psV", bufs=4, space="PSUM") as psV,
            ):
                wva = pWv.tile([128, 8, 512], f32r, tag="wva")
                wvb = pWv.tile([128, 8, 512], f32r, tag="wvb")
                nc.sync.dma_start(wva[:], w_t(WVA))
                nc.sync.dma_start(wvb[:], w_t(WVB))
                # ones columns for the rowsum trick (col 130h+64); DMA once
                for h in range(HPG):
                    nc.sync.dma_start(VA[:, :, 130 * h + 64:130 * h + 65],
                                      VONES[:, 16 * h:16 * h + 16, None])
                CW = 256
                for c in range(L // CW):
                    lsl = ts(c, CW)
                    xr = pX2.tile([128, 8, CW], f32r, tag="xr2")
                    xi = pX2.tile([128, 8, CW], f32r, tag="xi2")
                    nc.sync.dma_start(xr[:], xtr_t[:, :, lsl])
                    nc.sync.dma_start(xi[:], xti_t[:, :, lsl])
                    for tt in range(CW // 128):
                        t = c * (CW // 128) + tt
                        ps = psV.tile([128, 512], f32, tag="psV")
                        for kk in range(8):
                            mm(ps[:], xr[:, kk, ts(tt, 128)], wva[:, kk, :],
                               start=(kk == 0), stop=False)
                        for kk in range(8):
                            mm(ps[:], xi[:, kk, ts(tt, 128)], wvb[:, kk, :],
                               start=False, stop=(kk == 7))
                        for h in range(HPG):
                            nc.vector.tensor_copy(
                                VA[:, t, 130 * h:130 * h + 64],
                                ps[:, 128 * h:128 * h + 64])
                            nc.vector.tensor_copy(
                                VA[:, t, 130 * h + 65:130 * h + 129],
                                ps[:, 128 * h + 64:128 * h + 128])

            # ---------------- Phase B: attention ----------------
            pOTc = tc.alloc_tile_pool(name="ot", bufs=1)
            OTR = pOTc.tile([128, 2, L], f32r, tag="otr")
            OTI = pOTc.tile([128, 2, L], f32r, tag="oti")
            with (
                tc.tile_pool(name="attn", bufs=1) as pA,
                tc.tile_pool(name="pt", bufs=6) as pPT,
                tc.tile_pool(name="rb", bufs=2) as pRB,
                tc.tile_pool(name="stage", bufs=2) as pSt,
                tc.tile_pool(name="psS", bufs=2, space="PSUM") as psS,
                tc.tile_pool(name="psO", bufs=2, space="PSUM") as psO,
                tc.tile_pool(name="wo", bufs=1) as pWO,
                tc.tile_pool(name="psC", bufs=1, space="PSUM") as psC,
                tc.tile_pool(name="ostage", bufs=2) as pOS,
            ):
                mask2 = pA.tile([128, 4, 512], f32, tag="mask2")
                nc.sync.dma_start(mask2[:], MASK2[:, :].rearrange("(s p) f -> p s f", p=128))
                for h in range(HPG):
                    ko, odd = h // 2, h % 2
                    for c in range(4):
                        csl = ts(c, 512)
                        por = psO.tile([128, 512], f32, tag="por")
                        poi = psO.tile([128, 512], f32, tag="poi")
                        ntl = 4 * c + 4
                        for ti, t in enumerate(range(ntl - 1, -1, -1)):
                            pss = psS.tile([128, 512], f32, tag="psS")
                            mm(pss[:], KTS[:, h, ts(t, 128)], QTS[:, h, csl],
                               start=True, stop=True)
                            pt = pPT.tile([128, 512], f32r, tag="pt")
                            nc.scalar.activation(pt[:], pss[:], EXPF,
                                                 scale=HD ** -0.5)
                            sdiag = t - 4 * c
                            if sdiag >= 0:
                                nc.vector.tensor_mul(pt[:], pt[:],
                                                     mask2[:, sdiag, :])
                            st, sp = (ti == 0), (ti == ntl - 1)
                            vvi = VA[:, t, 130 * h + 65:130 * h + 129]
                            mm(por[0:65], VA[:, t, 130 * h:130 * h + 65],
                               pt[:], start=st, stop=sp)
                            mm(poi[0:64], vvi, pt[:], start=st, stop=sp)
                        recip = pRB.tile([1, 512], f32, tag="recip")
                        nc.vector.reciprocal(recip[:], por[64:65, :])
                        rb = pRB.tile([128, 512], f32, tag="rb")
                        nc.gpsimd.partition_broadcast(rb[:], recip[:])
                        if odd:
                            str_ = pSt.tile([64, 512], f32r, tag="str")
                            sti_ = pSt.tile([64, 512], f32r, tag="sti")
                            nc.vector.tensor_mul(str_[:], por[0:64, :],
                                                 rb[0:64, :])
                            nc.vector.tensor_mul(sti_[:], poi[0:64, :],
                                                 rb[0:64, :])
                            nc.sync.dma_start(OTR[64:128, ko, csl], str_[:])
                            nc.sync.dma_start(OTI[64:128, ko, csl], sti_[:])
                        else:
                            nc.vector.tensor_mul(OTR[0:64, ko, csl],
                                                 por[0:64, :], rb[0:64, :])
                            nc.vector.tensor_mul(OTI[0:64, ko, csl],
                                                 poi[0:64, :], rb[0:64, :])

                # ---------------- Phase C: output projection ----------------
                wor = pWO.tile([128, 2, D], f32r, tag="wor")
                woi = pWO.tile([128, 2, D], f32r, tag="woi")
                nwoi = pWO.tile([128, 2, D], f32r, tag="nwoi")
                nc.sync.dma_start(wor[:], wo_t(WOR))
                nc.sync.dma_start(woi[:], wo_t(WOI))
                nc.sync.dma_start(nwoi[:], wo_t(NWOI))
                for lt in range(16):
                    for j in range(2):
                        jsl = ts(j, 512)
                        pr = psC.tile([128, 512], f32, tag="pcr")
                        mm(pr[:], OTR[:, 0, ts(lt, 128)], wor[:, 0, jsl],
                           start=True, stop=False)
                        mm(pr[:], OTR[:, 1, ts(lt, 128)], wor[:, 1, jsl],
                           start=False, stop=False)
                        mm(pr[:], OTI[:, 0, ts(lt, 128)], nwoi[:, 0, jsl],
                           start=False, stop=False)
                        mm(pr[:], OTI[:, 1, ts(lt, 128)], nwoi[:, 1, jsl],
                           start=False, stop=True)
                        so = pOS.tile([128, 512], f32, tag="so")
                        nc.vector.tensor_copy(so[:], pr[:])
                        nc.sync.dma_start(OUTR[ts(lt, 128), jsl], so[:])

                        pi = psC.tile([128, 512], f32, tag="pci")
                        mm(pi[:], OTR[:, 0, ts(lt, 128)], woi[:, 0, jsl],
                           start=True, stop=False)
                        mm(pi[:], OTR[:, 1, ts(lt, 128)], woi[:, 1, jsl],
                           start=False, stop=False)
                        mm(pi[:], OTI[:, 0, ts(lt, 128)], wor[:, 0, jsl],
                           start=False, stop=False)
                        mm(pi[:], OTI[:, 1, ts(lt, 128)], wor[:, 1, jsl],
                           start=False, stop=True)
                        si = pOS.tile([128, 512], f32, tag="si")
                        nc.vector.tensor_copy(si[:], pi[:])
                        nc.sync.dma_start(OUTI[ts(lt, 128), jsl], si[:])
            pOTc.release()
            pV.release()
    nc.finalize()
    return nc


def kernel(x_real, x_imag, wq_r, wq_i, wk_r, wk_i, wv_r, wv_i, wo_r, wo_i,
           bo_r, bo_i, _trace=False):
    from concourse.bass_utils import run_bass_kernel_spmd

    f = np.float32
    x_real = np.asarray(x_real, f)
    x_imag = np.asarray(x_imag, f)
    wq_r, wq_i = np.asarray(wq_r, f), np.asarray(wq_i, f)
    wk_r, wk_i = np.asarray(wk_r, f), np.asarray(wk_i, f)
    wv_r, wv_i = np.asarray(wv_r, f), np.asarray(wv_i, f)
    wo_r, wo_i = np.asarray(wo_r, f), np.asarray(wo_i, f)

    cosT2, sinT2 = _rope_tables()
    maskt = _mask_tile()
    permt = _perm_matrix()

    in_maps = []
    for core in range(8):
        b, g = core // G, core % G
        cols = slice(g * GD, (g + 1) * GD)
        wva_blocks, wvb_blocks = [], []
        for h in range(HPG):
            r = slice((g * HPG + h) * HD, (g * HPG + h + 1) * HD)
            wva_blocks.append(np.concatenate([wv_r[r].T, wv_i[r].T], 1))
            wvb_blocks.append(np.concatenate([-wv_i[r].T, wv_r[r].T], 1))
        in_maps.append({
            "xtr": np.ascontiguousarray(x_real[b].T),
            "xti": np.ascontiguousarray(x_imag[b].T),
            "wq1": _stack_qk(wq_r, wq_i, g),
            "wq2": _stack_qk(-wq_i, wq_r, g),
            "wk1": _stack_qk(wk_r, wk_i, g),
            "wk2": _stack_qk(-wk_i, wk_r, g),
            "wva": np.ascontiguousarray(np.concatenate(wva_blocks, 1)),
            "wvb": np.ascontiguousarray(np.concatenate(wvb_blocks, 1)),
            "wor": np.ascontiguousarray(wo_r[:, cols].T),
            "woi": np.ascontiguousarray(wo_i[:, cols].T),
            "nwoi": np.ascontiguousarray(-wo_i[:, cols].T),
            "cos2": cosT2, "sin2": sinT2, "mask2": maskt,
            "perm": permt, "vones": np.ones((128, 128), np.float32),
        })

    key = "nc"
    if key not in _compiled:
        _compiled[key] = _build_kernel()
    try:
        res = run_bass_kernel_spmd(_compiled[key], in_maps,
                                   core_ids=list(range(8)), trace=_trace)
    except ModuleNotFoundError:
        res = run_bass_kernel_spmd(_compiled[key], in_maps,
                                   core_ids=list(range(8)), trace=False)
    kernel._last_results = res

    out_r = np.zeros((B, L, D), f)
    out_i = np.zeros((B, L, D), f)
    for core in range(8):
        b = core // G
        out_r[b] += res.results[core]["outr"]
        out_i[b] += res.results[core]["outi"]
    out_r += np.asarray(bo_r, f)
    out_i += np.asarray(bo_i, f)
    return out_r, out_i

